# revision 11
# baseline (speedup 1.0000x reference)
"""Trainium2 Bass kernel for nn_PhoenixRetrievalModel (retrieval_knn).

Distribution (8 NeuronCores, SPMD, one program / per-core inputs):
  - User tower: data-parallel over batch (8 rows/core = 1280 tokens), fp32,
    then AllGather so every core holds all 64 user vectors.
  - Corpus: sharded 62500 items/core on the N axis. Each core streams its
    shard from HBM, PE-transposes 128x128 blocks, computes
    sims = user @ corpusT in fp32 (PSUM accumulation over 4 K-chunks plus a
    K=1 penalty matmul that applies corpus_mask as an additive -1e12), and
    reduces every 256-item tile to its top-8 (values + indices) with DVE
    max8 / max_index.
  - Host: gathers the 8 partial candidate lists and re-reduces to the exact
    global top-256 per row ordered by (score desc, index asc) — same
    semantics as jax.lax.top_k.

Returns (user_representation [64,512] f32, top_k_indices [64,256] i32,
top_k_scores [64,256] f32).
"""
import numpy as np

import concourse.bass as bass
import concourse.mybir as mybir
import concourse.tile as tile
from concourse.masks import make_identity
from concourse.bass_utils import run_bass_kernel_spmd

# walrus in this environment supports a limited number of embedded sync-wait
# commands per instruction struct; split the excess into standalone
# EventSemaphore waits on the same engine.
_WAIT_CAPS = {
    "InstDMACopy": 1,
    "InstMatmult": 1,
    "InstEventSemaphore": 1000,
    "InstCall": 1000,
}


def _legalize_waits(nc):
    for f in nc.m.functions:
        for blk in f.blocks:
            new = []
            changed = False
            for inst in blk.instructions:
                si = inst.sync_info
                cap = _WAIT_CAPS.get(type(inst).__name__, 1)
                if si is not None and si.on_wait and len(si.on_wait) > cap:
                    waits = list(si.on_wait)
                    excess, keep = waits[:-cap], waits[-cap:]
                    for j, w in enumerate(excess):
                        ev = mybir.InstEventSemaphore(
                            name=f"{inst.name}_w{j}", ins=[], outs=[])
                        ev.engine = inst.engine
                        ev.sync_info = mybir.SyncInfo(on_wait=[w], on_update=[])
                        new.append(ev)
                    inst.sync_info = mybir.SyncInfo(
                        on_wait=keep, on_update=list(si.on_update))
                    changed = True
                new.append(inst)
            if changed:
                blk.instructions = new

F32 = mybir.dt.float32
BF16 = mybir.dt.bfloat16
U16 = mybir.dt.uint16
AF = mybir.ActivationFunctionType
AX = mybir.AxisListType

NCORES = 8
B, S, D = 64, 160, 512
NH, DH = 8, 64
BPC = B // NCORES            # batch rows per core
TOK = BPC * S                # tokens per core (1280)
NTT = TOK // 128             # token tiles (10)
KC = D // 128                # K chunks of the model dim (4)
INF = 1e12
TCH = (512, 512, 256)        # token chunking for 512-wide PSUM


def _geom(nsh):
    """Per-core corpus geometry: pad shard to a multiple of 2048, two halves."""
    npad = ((nsh + 2047) // 2048) * 2048
    nsup = npad // 1024                       # supertiles of 1024 items
    nreg = nsup // 2                          # sims regions [128p, 1024]
    half = nreg * 1024                        # padded items per half
    return npad, nsup, nreg, half


def build_bass(nsh):
    npad, nsup, nreg, half = _geom(nsh)
    ncand = (half // 256) * 8                 # candidate slots per partition

    nc = bass.Bass("TRN2", target_bir_lowering=False, debug=False,
                   num_devices=NCORES)

    emb_d = nc.dram_tensor("emb", [TOK, D], F32, kind="ExternalInput")
    pmf_d = nc.dram_tensor("pmf", [1, TOK], F32, kind="ExternalInput")
    poolm_d = nc.dram_tensor("poolm", [TOK, BPC], F32, kind="ExternalInput")
    wq_d = nc.dram_tensor("wq", [D, D], F32, kind="ExternalInput")
    wk_d = nc.dram_tensor("wk", [D, D], F32, kind="ExternalInput")
    wv_d = nc.dram_tensor("wv", [D, D], F32, kind="ExternalInput")
    wo_d = nc.dram_tensor("wo", [D, D], F32, kind="ExternalInput")
    w1_d = nc.dram_tensor("w1", [D, 4 * D], F32, kind="ExternalInput")
    w2_d = nc.dram_tensor("w2", [4 * D, D], F32, kind="ExternalInput")
    corp_d = nc.dram_tensor("corp", [nsh, D], F32, kind="ExternalInput")
    pen_d = nc.dram_tensor("pen", [nsup, 1024], F32, kind="ExternalInput")
    msum_d = nc.dram_tensor("msum", [BPC, 1], F32, kind="ExternalInput")

    user_d = nc.dram_tensor("user", [B, D], F32, kind="ExternalOutput")
    cval_d = nc.dram_tensor("cval", [128, ncand], F32, kind="ExternalOutput")
    cidx_d = nc.dram_tensor("cidx", [128, ncand], U16, kind="ExternalOutput")

    with tile.TileContext(nc) as tc:
        with (
            tc.tile_pool(name="const", bufs=1) as constp,
            tc.tile_pool(name="dram", bufs=1, space="DRAM") as dramp,
        ):
            ident = constp.tile([128, 128], F32)
            make_identity(nc, ident)
            ones1 = constp.tile([1, 128], BF16)
            nc.vector.memset(ones1, 1.0)
            userT_lo = constp.tile([128, KC * 128], F32)
            userT_hi = constp.tile([128, KC * 128], F32)
            user_all = constp.tile([B, D], F32)

            _build_tower(nc, tc, dramp, ident, ones1, userT_lo, userT_hi,
                         user_all, emb_d, pmf_d, poolm_d, msum_d,
                         (wq_d, wk_d, wv_d, wo_d, w1_d, w2_d), user_d)
            _build_corpus(nc, tc, ident, ones1, userT_lo, userT_hi,
                          corp_d, pen_d, cval_d, cidx_d, nsh, nsup, nreg)
    _legalize_waits(nc)
    return nc


def _build_tower(nc, tc, dramp, ident, ones1, userT_lo, userT_hi, user_all,
                 emb_d, pmf_d, poolm_d, msum_d, wdrams, user_d):
    wq_d, wk_d, wv_d, wo_d, w1_d, w2_d = wdrams
    with (
        tc.tile_pool(name="tw", bufs=1) as tw,
        tc.tile_pool(name="tws", bufs=2) as tws,
        tc.tile_pool(name="ps_tp", bufs=2, space="PSUM") as ps_tp,
        tc.tile_pool(name="ps_sp", bufs=2, space="PSUM") as ps_sp,
        tc.tile_pool(name="ps_misc", bufs=1, space="PSUM") as ps_misc,
        tc.tile_pool(name="ps_acc", bufs=2, space="PSUM") as ps_acc,
    ):
        # tag-shared slots (sequential lifetimes):
        #  A: emb_td -> qT -> gT          (20K)
        #  B: h_td -> emb2 -> h2_td       (20K)
        #  C: hT -> h2T                   (20K)
        #  D: kT -> x2_td                 (20K)
        #  E: oT -> w2                    (32K)
        #  F: x_td                        (20K)
        #  G: wqkvo -> w1                 (32K)
        emb_td = tw.tile([128, NTT * D], F32, tag="A")
        nc.gpsimd.dma_start(
            emb_td[:].rearrange("p (t d) -> p t d", t=NTT),
            emb_d.ap().rearrange("(t p) d -> p t d", p=128))
        pmf = tw.tile([1, TOK], BF16, tag="pmf")
        pmf32 = tw.tile([1, TOK], F32, tag="pmf32")
        nc.gpsimd.dma_start(pmf32[:], pmf_d.ap())
        nc.vector.tensor_copy(pmf[:], pmf32[:])
        poolm = tw.tile([128, NTT * BPC], F32, tag="poolm")
        nc.gpsimd.dma_start(
            poolm[:].rearrange("p (t b) -> p t b", t=NTT),
            poolm_d.ap().rearrange("(t p) b -> p t b", p=128))

        wqkvo = tw.tile([128, 4 * KC * D], F32, tag="G")
        for i, wd in enumerate((wq_d, wk_d, wv_d, wo_d)):
            nc.gpsimd.dma_start(
                wqkvo[:, i * KC * D:(i + 1) * KC * D]
                .rearrange("p (k m) -> p k m", k=KC),
                wd.ap().rearrange("(k p) m -> p k m", p=128))
        wq = wqkvo[:, 0 * KC * D:1 * KC * D]
        wk = wqkvo[:, 1 * KC * D:2 * KC * D]
        wv = wqkvo[:, 2 * KC * D:3 * KC * D]
        wo = wqkvo[:, 3 * KC * D:4 * KC * D]

        epsb = tw.tile([128, 1], F32, tag="epsb")
        nc.vector.memset(epsb, 1e-6)

        def rmsnorm(dst, src):
            # dst = src * rsqrt(mean(src^2) + 1e-6); dst also used as scratch
            for t in range(NTT):
                ssq = tws.tile([128, 1], F32, tag="ssq")
                nc.scalar.activation(dst[:, t * D:(t + 1) * D],
                                     src[:, t * D:(t + 1) * D],
                                     AF.Square, accum_out=ssq[:])
                rs = tws.tile([128, 1], F32, tag="rs")
                nc.scalar.activation(rs[:], ssq[:], AF.Sqrt,
                                     scale=1.0 / D, bias=epsb[:])
                nc.vector.reciprocal(rs[:], rs[:])
                nc.vector.tensor_scalar_mul(
                    dst[:, t * D:(t + 1) * D], src[:, t * D:(t + 1) * D], rs[:])

        def transpose_td(dstT, src):
            for t in range(NTT):
                for k in range(KC):
                    tp = ps_tp.tile([128, 128], F32, tag="tp")
                    nc.tensor.transpose(
                        tp[:], src[:, t * D + k * 128: t * D + (k + 1) * 128],
                        ident[:])
                    nc.scalar.copy(
                        dstT[:, k * TOK + t * 128: k * TOK + (t + 1) * 128], tp[:])

        h_td = tw.tile([128, NTT * D], F32, tag="B")
        rmsnorm(h_td, emb_td)
        hT = tw.tile([128, KC * TOK], F32, tag="C")
        transpose_td(hT, h_td)

        # --- Q, K projections (transposed layout) ---
        qT = tw.tile([128, KC * TOK], F32, tag="A")
        kT = tw.tile([128, KC * TOK], F32, tag="D")
        for dst, w in ((qT, wq), (kT, wk)):
            for c in range(KC):
                off = 0
                for tch in TCH:
                    acc = ps_acc.tile([128, 512], F32, tag="acc")
                    for k in range(KC):
                        nc.tensor.matmul(
                            acc[:, :tch],
                            w[:, k * D + c * 128: k * D + (c + 1) * 128],
                            hT[:, k * TOK + off: k * TOK + off + tch],
                            start=(k == 0), stop=(k == KC - 1))
                    nc.scalar.copy(dst[:, c * TOK + off: c * TOK + off + tch],
                                   acc[:, :tch])
                    off += tch

        # --- attention (V computed per batch) ---
        oT = tw.tile([128, KC * TOK], F32, tag="E")
        scale = DH ** -0.5
        for b in range(BPC):
            v_a = tws.tile([128, D], F32, tag="v_a")   # tokens b*160+[0,128)
            v_b = tws.tile([128, D], F32, tag="v_b")   # tokens b*160+[128,160)
            for (dst, toff, tl) in ((v_a, 0, 128), (v_b, 128, 32)):
                acc = ps_acc.tile([128, 512], F32, tag="acc")
                for k in range(KC):
                    nc.tensor.matmul(
                        acc[:tl, :],
                        hT[:, k * TOK + b * S + toff: k * TOK + b * S + toff + tl],
                        wv[:, k * D: (k + 1) * D],
                        start=(k == 0), stop=(k == KC - 1))
                nc.scalar.copy(dst[:tl, :], acc[:tl, :])
            kmp = ps_misc.tile([128, S], F32, tag="km")
            nc.tensor.matmul(kmp[:], ones1[:], pmf[:, b * S:(b + 1) * S],
                             start=True, stop=True)
            kmask = tws.tile([128, S], F32, tag="kmask")
            nc.scalar.copy(kmask[:], kmp[:])
            for h in range(NH):
                c, po = h // 2, (h % 2) * 64
                q_bh = qT[po:po + 64, c * TOK + b * S: c * TOK + (b + 1) * S]
                k_bh = kT[po:po + 64, c * TOK + b * S: c * TOK + (b + 1) * S]
                attT_a = tws.tile([128, S], F32, tag="attT_a")
                attT_b = tws.tile([128, S], F32, tag="attT_b")
                for (qo, ql) in ((0, 128), (128, 32)):
                    sp = ps_sp.tile([128, S], F32, tag="sp")
                    nc.tensor.matmul(sp[:ql, :], q_bh[:, qo:qo + ql], k_bh[:],
                                     start=True, stop=True)
                    s_sb = tws.tile([128, S], F32, tag="s_sb")
                    nc.scalar.activation(s_sb[:ql, :], sp[:ql, :], AF.Copy,
                                         scale=scale)
                    rmax = tws.tile([128, 1], F32, tag="rmax")
                    nc.vector.reduce_max(rmax[:ql], s_sb[:ql, :], axis=AX.X)
                    nc.vector.tensor_scalar_mul(rmax[:ql], rmax[:ql], -1.0)
                    e = tws.tile([128, S], F32, tag="e")
                    nc.scalar.activation(e[:ql, :], s_sb[:ql, :], AF.Exp,
                                         bias=rmax[:ql], scale=1.0)
                    nc.vector.tensor_mul(e[:ql, :], e[:ql, :], kmask[:ql, :])
                    den = tws.tile([128, 1], F32, tag="den")
                    nc.vector.reduce_sum(den[:ql], e[:ql, :], axis=AX.X)
                    nc.vector.reciprocal(den[:ql], den[:ql])
                    nc.vector.tensor_scalar_mul(e[:ql, :], e[:ql, :], den[:ql])
                    for (dstT, ko, kl) in ((attT_a, 0, 128), (attT_b, 128, 32)):
                        tp = ps_tp.tile([128, 128], F32, tag="tp")
                        nc.tensor.transpose(tp[:kl, :ql], e[:ql, ko:ko + kl],
                                            ident[:ql, :ql])
                        nc.scalar.copy(dstT[:kl, qo:qo + ql], tp[:kl, :ql])
                op = ps_sp.tile([128, S], F32, tag="sp")
                nc.tensor.matmul(op[po:po + 64, :],
                                 v_a[:, h * DH:(h + 1) * DH],
                                 attT_a[:], start=True, stop=False,
                                 skip_group_check=True)
                nc.tensor.matmul(op[po:po + 64, :],
                                 v_b[:32, h * DH:(h + 1) * DH],
                                 attT_b[:32, :], start=False, stop=True,
                                 skip_group_check=True)
                nc.scalar.copy(oT[po:po + 64, c * TOK + b * S: c * TOK + (b + 1) * S],
                               op[po:po + 64, :])

        # --- out proj + residual (emb re-loaded into slot B) ---
        emb2 = tw.tile([128, NTT * D], F32, tag="B")
        nc.gpsimd.dma_start(
            emb2[:].rearrange("p (t d) -> p t d", t=NTT),
            emb_d.ap().rearrange("(t p) d -> p t d", p=128))
        x_td = tw.tile([128, NTT * D], F32, tag="F")
        for t in range(NTT):
            acc = ps_acc.tile([128, 512], F32, tag="acc")
            for k in range(KC):
                nc.tensor.matmul(acc[:],
                                 oT[:, k * TOK + t * 128: k * TOK + (t + 1) * 128],
                                 wo[:, k * D:(k + 1) * D],
                                 start=(k == 0), stop=(k == KC - 1))
            nc.vector.tensor_add(x_td[:, t * D:(t + 1) * D], acc[:],
                                 emb2[:, t * D:(t + 1) * D])

        # --- MLP (256-token chunks) ---
        w1 = tw.tile([128, KC * 4 * D], F32, tag="G")
        nc.gpsimd.dma_start(w1[:].rearrange("p (k m) -> p k m", k=KC),
                          w1_d.ap().rearrange("(k p) m -> p k m", p=128))
        w2 = tw.tile([128, 16 * D], F32, tag="E")
        nc.gpsimd.dma_start(w2[:].rearrange("p (k m) -> p k m", k=16),
                          w2_d.ap().rearrange("(k p) m -> p k m", p=128))
        h2_td = tw.tile([128, NTT * D], F32, tag="B")
        rmsnorm(h2_td, x_td)
        h2T = tw.tile([128, KC * TOK], F32, tag="C")
        transpose_td(h2T, h2_td)
        x2_td = tw.tile([128, NTT * D], F32, tag="D")
        MCH = 256
        gT = tw.tile([128, 16 * MCH], F32, tag="A")
        for mc in range(TOK // MCH):
            off = mc * MCH
            for gc in range(16):
                acc = ps_acc.tile([128, 512], F32, tag="acc")
                for k in range(KC):
                    nc.tensor.matmul(
                        acc[:, :MCH],
                        w1[:, k * 4 * D + gc * 128: k * 4 * D + (gc + 1) * 128],
                        h2T[:, k * TOK + off: k * TOK + off + MCH],
                        start=(k == 0), stop=(k == KC - 1))
                nc.scalar.activation(gT[:, gc * MCH:(gc + 1) * MCH],
                                     acc[:, :MCH], AF.Sigmoid)
                nc.vector.tensor_mul(gT[:, gc * MCH:(gc + 1) * MCH],
                                     gT[:, gc * MCH:(gc + 1) * MCH], acc[:, :MCH])
            for tl in range(MCH // 128):
                t = off // 128 + tl
                acc = ps_acc.tile([128, 512], F32, tag="acc")
                for gc in range(16):
                    nc.tensor.matmul(
                        acc[:],
                        gT[:, gc * MCH + tl * 128: gc * MCH + (tl + 1) * 128],
                        w2[:, gc * D:(gc + 1) * D],
                        start=(gc == 0), stop=(gc == 15))
                nc.vector.tensor_add(x2_td[:, t * D:(t + 1) * D], acc[:],
                                     x_td[:, t * D:(t + 1) * D])

        # --- masked mean pool + l2 norm ---
        up = ps_acc.tile([128, 512], F32, tag="acc")
        for t in range(NTT):
            nc.tensor.matmul(up[:BPC, :], poolm[:, t * BPC:(t + 1) * BPC],
                             x2_td[:, t * D:(t + 1) * D],
                             start=(t == 0), stop=(t == NTT - 1))
        msum = tws.tile([BPC, 1], F32, tag="msum")
        nc.gpsimd.dma_start(msum[:], msum_d.ap())
        user8 = tws.tile([BPC, D], F32, tag="user8")
        mrec = tws.tile([BPC, 1], F32, tag="mrec")
        nc.vector.tensor_scalar_max(mrec[:], msum[:BPC, :], 1.0)
        nc.vector.reciprocal(mrec[:], mrec[:])
        nc.vector.tensor_scalar_mul(user8[:], up[:BPC, :], mrec[:])
        ssq = tws.tile([BPC, 1], F32, tag="ssq8")
        sqt = tws.tile([BPC, D], F32, tag="sq8")
        nc.scalar.activation(sqt[:], user8[:], AF.Square, accum_out=ssq[:])
        nc.vector.tensor_scalar_max(ssq[:], ssq[:], 1e-12)
        nc.scalar.activation(ssq[:], ssq[:], AF.Sqrt)
        nc.vector.reciprocal(ssq[:], ssq[:])
        nc.vector.tensor_scalar_mul(user8[:], user8[:], ssq[:])

        # --- AllGather users across the 8 cores ---
        agin = dramp.tile([BPC, D], F32)
        agout = dramp.tile([B, D], F32)
        nc.gpsimd.dma_start(agin[:], user8[:])
        nc.gpsimd.collective_compute(
            "AllGather", mybir.AluOpType.bypass,
            replica_groups=[list(range(NCORES))],
            ins=[agin.opt()], outs=[agout.opt()])
        nc.gpsimd.dma_start(user_all[:], agout[:])
        nc.gpsimd.dma_start(user_d.ap(), user_all[:])

        # stationaries [128, KC*128]: lo = rows in cols 0:64, hi = cols 64:128
        nc.vector.memset(userT_lo, 0.0)
        nc.vector.memset(userT_hi, 0.0)
        for k in range(KC):
            tp = ps_tp.tile([128, 128], F32, tag="tp")
            nc.tensor.transpose(tp[:, :64], user_all[:, k * 128:(k + 1) * 128],
                                ident[:64, :64])
            nc.scalar.copy(userT_lo[:, k * 128: k * 128 + 64], tp[:, :64])
            nc.scalar.copy(userT_hi[:, k * 128 + 64: (k + 1) * 128], tp[:, :64])


def _build_corpus(nc, tc, ident, ones1, userT_lo, userT_hi,
                  corp_d, pen_d, cval_d, cidx_d, nsh, nsup, nreg):
    ncand = (nreg * 1024 // 256) * 8
    with (
        tc.tile_pool(name="cands", bufs=1) as candp,
        tc.tile_pool(name="csb", bufs=3) as csbp,
        tc.tile_pool(name="ctb", bufs=3) as ctbp,
        tc.tile_pool(name="penp", bufs=2) as penp,
        tc.tile_pool(name="simsp", bufs=3) as simsp,
        tc.tile_pool(name="cps_tp", bufs=6, space="PSUM") as cps_tp,
        tc.tile_pool(name="cps_acc", bufs=2, space="PSUM") as cps_acc,
    ):
        cval = candp.tile([128, ncand], F32)
        cidx = candp.tile([128, ncand], U16)

        def supertile(g, hi, sims, r):
            csb = csbp.tile([128, 8 * D], F32, tag="csb")
            base = g * 1024
            nvalid = max(0, min(1024, nsh - base))
            nj = nvalid // 128
            if nj > 0:
                nc.gpsimd.dma_start(
                    csb[:, :nj * D].rearrange("p (j d) -> p j d", j=nj),
                    corp_d.ap()[base:base + nj * 128]
                    .rearrange("(j p) d -> p j d", p=128))
            rem = nvalid - nj * 128
            if rem > 0:
                nc.gpsimd.dma_start(csb[:rem, nj * D:(nj + 1) * D],
                                  corp_d.ap()[base + nj * 128: base + nvalid])
            pen_sb = penp.tile([1, 1024], F32, tag="pen")
            nc.gpsimd.dma_start(pen_sb[:], pen_d.ap()[g:g + 1, :])
            pen_bf = penp.tile([1, 1024], BF16, tag="penb")
            nc.vector.tensor_copy(pen_bf[:], pen_sb[:])

            lhs = userT_hi if hi else userT_lo
            po = 64 if hi else 0
            for ch in range(2):
                ctb = ctbp.tile([128, KC * 512], F32, tag="ctb")
                for j in range(4):
                    jj = ch * 4 + j
                    for k in range(KC):
                        tp = cps_tp.tile([128, 128], F32, tag="tp")
                        nc.tensor.transpose(
                            tp[:], csb[:, jj * D + k * 128: jj * D + (k + 1) * 128],
                            ident[:])
                        nc.scalar.copy(
                            ctb[:, k * 512 + j * 128: k * 512 + (j + 1) * 128], tp[:])
                acc = cps_acc.tile([128, 512], F32, tag="acc")
                for k in range(KC):
                    nc.tensor.matmul(acc[:], lhs[:, k * 128:(k + 1) * 128],
                                     ctb[:, k * 512:(k + 1) * 512],
                                     start=(k == 0), stop=False)
                nc.tensor.matmul(acc[:], ones1[:],
                                 pen_bf[:, ch * 512:(ch + 1) * 512],
                                 start=False, stop=True)
                nc.scalar.copy(sims[po:po + 64, ch * 512:(ch + 1) * 512],
                               acc[po:po + 64, :])
                if hi:
                    for tt in range(2):
                        t = r * 4 + ch * 2 + tt
                        fo = ch * 512 + tt * 256
                        nc.vector.max(cval[:, t * 8:(t + 1) * 8],
                                      sims[:, fo:fo + 256])
                        nc.vector.max_index(cidx[:, t * 8:(t + 1) * 8],
                                            cval[:, t * 8:(t + 1) * 8],
                                            sims[:, fo:fo + 256])
                        if t > 0:
                            nc.vector.tensor_scalar_add(
                                cidx[:, t * 8:(t + 1) * 8],
                                cidx[:, t * 8:(t + 1) * 8], t * 256)

        for r in range(nreg):
            sims = simsp.tile([128, 1024], F32, tag="sims")
            supertile(r, False, sims, r)
            supertile(nreg + r, True, sims, r)

        nc.gpsimd.dma_start(cval_d.ap(), cval[:])
        nc.gpsimd.dma_start(cidx_d.ap(), cidx[:])


# ---------------------------------------------------------------------------
# host side
# ---------------------------------------------------------------------------

def shard_inputs(inputs, nsh):
    npad, nsup, nreg, half = _geom(nsh)
    emb = np.ascontiguousarray(np.asarray(inputs["embeddings"], dtype=np.float32))
    pm = np.asarray(inputs["padding_mask"]).astype(np.float32)
    corpus = np.ascontiguousarray(np.asarray(inputs["corpus_embeddings"],
                                             dtype=np.float32))
    cmask = np.asarray(inputs["corpus_mask"]).astype(np.float32)
    ws = {k: np.ascontiguousarray(np.asarray(inputs[K], dtype=np.float32))
          for k, K in (("wq", "Wq"), ("wk", "Wk"), ("wv", "Wv"), ("wo", "Wo"),
                       ("w1", "W1"), ("w2", "W2"))}
    in_maps = []
    for c in range(NCORES):
        emb_c = emb[c * BPC:(c + 1) * BPC].reshape(TOK, D)
        pmf_c = pm[c * BPC:(c + 1) * BPC]
        poolm = np.zeros((TOK, BPC), np.float32)
        for b in range(BPC):
            poolm[b * S:(b + 1) * S, b] = pmf_c[b]
        sh = np.ascontiguousarray(corpus[c * nsh:(c + 1) * nsh])
        mk = cmask[c * nsh:(c + 1) * nsh]
        pen = np.full(npad, -INF, np.float32)
        pen[:nsh] = (mk - 1.0) * INF
        in_maps.append(dict(emb=emb_c, pmf=pmf_c.reshape(1, TOK), poolm=poolm,
                            msum=pmf_c.sum(1, keepdims=True).astype(np.float32),
                            corp=sh, pen=pen.reshape(nsup, 1024), **ws))
    return in_maps


def merge(results, nsh, top_k=256):
    npad, nsup, nreg, half = _geom(nsh)
    vals, idxs = [], []
    for c, res in enumerate(results):
        cv = np.asarray(res["cval"])
        ci = np.asarray(res["cidx"]).astype(np.int64)
        v = np.concatenate([cv[:64], cv[64:]], axis=1)
        g = np.concatenate([ci[:64] + c * nsh, ci[64:] + half + c * nsh], axis=1)
        vals.append(v)
        idxs.append(g)
    V = np.concatenate(vals, axis=1)
    G = np.concatenate(idxs, axis=1)
    out_idx = np.zeros((B, top_k), np.int32)
    out_val = np.zeros((B, top_k), np.float32)
    kk = min(2 * top_k, V.shape[1] - 1)
    for r in range(B):
        v, g = V[r], G[r]
        sel = np.argpartition(-v, kk)[:kk]
        o = sel[np.lexsort((g[sel], -v[sel]))][:top_k]
        out_idx[r] = g[o].astype(np.int32)
        out_val[r] = v[o]
    return out_idx, out_val


_BUILT = {}


def kernel(**inputs):
    n = int(inputs["corpus_embeddings"].shape[0])
    assert n % NCORES == 0
    nsh = n // NCORES
    if nsh not in _BUILT:
        _BUILT[nsh] = build_bass(nsh)
    nc = _BUILT[nsh]
    in_maps = shard_inputs(inputs, nsh)
    r = run_bass_kernel_spmd(nc, in_maps, core_ids=list(range(NCORES)))
    user = np.asarray(r.results[0]["user"])
    top_k = int(inputs.get("top_k", 256))
    idx, val = merge(r.results, nsh, top_k)
    return user, idx, val
